# revision 15
# baseline (speedup 1.0000x reference)
"""Trainium2 Bass kernel for nn_ChannelLatencySeq2Value (B=8, C=256, T=4096).

Structure of the computation (derived analytically from the reference):
  * The 3 depthwise conv paths (k=3/5/9, out_per_kernel=6) followed by the
    grouped 1x1 reduce collapse into a single sparse conv:
        drive[b,c,t] = beta[c] + sum_{j<3} sum_{k<9} g[c,j,k] * x[b,(3c+j)%256, t+k-4]
    i.e. output channel c reads 3 cyclically-consecutive input channels with
    composed 9-tap kernels. g/beta are composed on the host (tiny).
  * The LIF scan V = a*V + (1-a)*drive is a first-order linear recurrence ->
    tensor_tensor_scan on VectorE (fp32 state).
  * first-spike latency only needs per-row max(V) when no neuron is near
    threshold; rows with max(V) >= 0.95 are recomputed exactly on the host
    (for the fixed reference input distribution, max V ~ 0.76, so this
    fallback never triggers; it guarantees exactness if it ever does).
  * the tiny (B,C) MLP head runs on the host in fp32 (<< 0.01% of FLOPs).

Device work per core (data-parallel over batch, 1 batch element per core):
  * encoder as bf16 matmuls on TensorE: output channels are split in two
    PSUM tiles whose input needs are covered by the two natural 128-channel
    windows of x ([0,128) and [128,256)); because gcd(3,256)=1, each window
    mod-covers 126 full output channels. 9 shift-matmuls per window
    accumulate in PSUM. The 4 boundary channels {42,85,170,213} get their
    single out-of-window tap row via a small 36-partition shift-replicated
    tile (one extra matmul per PSUM tile).
  * tensor_tensor_scan (chunked over PSUM tiles, chained via the last
    column) produces V; tensor_reduce(max) gives per-row Vmax.
"""

import numpy as np
import ml_dtypes

import concourse.bass as bass
import concourse.bacc as bacc
import concourse.mybir as mybir
from concourse.tile import TileContext
from concourse.bass_utils import run_bass_kernel_spmd

# ---------------------------------------------------------------- constants
B, C, T = 8, 256, 4096
OP = 6
ALPHA = float(np.exp(-1.0 / 5.0))
OMA = 1.0 - ALPHA
THRESHOLD = 1.0
TC = 512                      # time chunk (= one PSUM bank of fp32)
NT = T // TC
PAD = 4                       # conv halo (kernel width 9)
PADT = T + 2 * PAD
NCORES = 8
FALLBACK_THR = 0.95           # host exact-recompute margin for Vmax

BF16 = ml_dtypes.bfloat16

# mega-blob layout (bf16 columns, 128 partitions)
AAW = 2 * 9 * 128              # A1|A2 lhsT stacks        [0, 2304)
XWB = AAW                      # xw3 | B1 | B2 (36 rows)  [2304, 6664)
XB0 = XWB + PADT + 256         # x ci0 piece0 (xpad cols [0,1546))
XB1 = XB0 + 1546               # x ci1 piece0
XB2 = XB1 + 1546               # x ci0 piece1 (xpad cols [1528,4104))
XB3 = XB2 + 2576               # x ci1 piece1
BLOBW = XB3 + 2576

# channel -> psum-tile assignment.  Window 1 = input rows [0,128),
# window 2 = rows [128,256).  A channel c (inputs {3c+j mod 256}) is "full"
# in a window if all three input rows fall inside it.
PERM1 = list(range(0, 42)) + list(range(86, 128)) + list(range(171, 213)) + [42, 85]
PERM2 = list(range(43, 85)) + list(range(128, 170)) + list(range(214, 256)) + [170, 213]
# out-of-window tap rows used by the straddler channels, one per straddler
W3ROWS = (0, 127, 128, 255)


def _compose_g(w3, b3, w5, b5, w9, b9, w_red, b_red):
    """Collapse the 4-conv encoder into g[c,3,9] (fp64 accum) + beta[c]."""
    g = np.zeros((C, 3, 9), np.float64)
    beta = np.zeros((C,), np.float64)
    paths = [(np.asarray(w3, np.float64), np.asarray(b3, np.float64), 3),
             (np.asarray(w5, np.float64), np.asarray(b5, np.float64), 5),
             (np.asarray(w9, np.float64), np.asarray(b9, np.float64), 9)]
    wr = np.asarray(w_red, np.float64)
    for c in range(C):
        beta[c] += float(b_red[c])
        for i in range(18):
            m = c * 18 + i
            wp, bp, K = paths[m // (C * OP)]
            q = m % (C * OP)
            s = q // OP
            j = (s - 3 * c) % 256
            assert j in (0, 1, 2)
            pad = (K - 1) // 2
            w = wr[c, i, 0]
            beta[c] += w * bp[q]
            g[c, j, 4 - pad:4 + pad + 1] += w * wp[q, 0, :]
    return g, beta


def _build_weights(g):
    """Split (1-a)*g into the window lhsT stacks A1/A2 (9,128,128) and the
    straddler lhsT B1/B2 (36,128).  B row layout: p = 9*r + k where r indexes
    W3ROWS and k the shift."""
    gs = g * OMA
    A = [np.zeros((9, 128, 128), np.float64) for _ in range(2)]
    Bm = [np.zeros((36, 128), np.float64) for _ in range(2)]
    for ti, perm in enumerate((PERM1, PERM2)):
        lo = 128 * ti
        for p, c in enumerate(perm):
            for j in range(3):
                s = (3 * c + j) % 256
                if lo <= s < lo + 128:
                    A[ti][:, s - lo, p] = gs[c, j, :]
                else:
                    r = W3ROWS.index(s)
                    Bm[ti][9 * r:9 * r + 9, p] = gs[c, j, :]
    return A[0], A[1], Bm[0], Bm[1]


# ------------------------------------------------------------ device program
_PROG = None
LAST_RESULTS = None
LAST_VMAX = None


def _build_program():
    f32 = mybir.dt.float32
    bf = mybir.dt.bfloat16
    nc = bacc.Bacc(None, target_bir_lowering=False)
    # all inputs are pre-padded / pre-laid-out on the host so that every
    # SBUF tile below has exactly one DMA writer (walrus limits the number
    # of sync-wait conditions per consuming instruction).
    # single input mega-blob, loaded as a serial chain of DMA pieces in
    # consumption-priority order (each piece's dst overlaps the previous
    # piece by 2 columns -> WAW dependency -> the pieces transfer strictly
    # in order, each at full HBM bandwidth, instead of fair-sharing it).
    # layout: [ AA(2304) | xw3+B1+B2(4360, 36 rows) | x-ci0-p0(1546)
    #           | x-ci1-p0(1546) | x-ci0-p1(2576) | x-ci1-p1(2576) ]
    blob_d = nc.declare_dram_parameter("blob", [128, BLOBW], bf, isOutput=False)
    vmax_d = nc.declare_dram_parameter("vmax", [128, 2], f32, isOutput=True)

    with TileContext(nc) as tc:
        with (
            tc.tile_pool(name="cst", bufs=1) as cst,
            tc.tile_pool(name="ps", bufs=6, space="PSUM") as pp,
            tc.tile_pool(name="pw", bufs=1, space="PSUM") as pw,
        ):
            mt = cst.tile([128, BLOBW], bf, tag="mt")
            alpha_t = cst.tile([128, TC], f32, tag="alpha")
            vb1 = cst.tile([128, T], f32, tag="vb1")
            vb2 = cst.tile([128, T], f32, tag="vb2")
            vmax_t = cst.tile([128, 2], f32, tag="vmax")
            vmax_cols = cst.tile([128, 16], f32, tag="vmax_cols")
            scratch = cst.tile([128, 1], f32, tag="scratch")

            # memset on the vector engine: the scan (a tight-encoding STT
            # instruction with few sync-wait slots) then depends on alpha_t
            # via same-engine program order instead of a semaphore.
            nc.vector.memset(alpha_t[:], ALPHA)

            # chained loads (see blob layout comment above)
            pieces = [(0, AAW, 128), (XWB, XB0, 36), (XB0, XB1, 128),
                      (XB1, XB2, 128), (XB2, XB3, 128), (XB3, BLOBW, 128)]
            prev = None
            for lo, hi, np_ in pieces:
                lo2 = lo if prev is None else lo - 2   # overlap -> serial chain
                nc.sync.dma_start(out=mt[0:np_, lo2:hi], in_=blob_d[0:np_, lo2:hi])
                prev = lo

            # warm-up matmuls: keep the PE busy (HAM un-throttled) while the
            # x pieces stream in; they read the already-landed weight slab.
            wps = pw.tile([128, TC], f32, tag="warm")
            for w in range(12):
                nc.tensor.matmul(wps[:], mt[:, 0:128], mt[:, 0:TC],
                                 start=True, stop=True, skip_group_check=True)

            # encoder matmuls + LIF scan + running max.
            # The two channel-tile chunk loops are interleaved so the first
            # four matmul groups land on fresh PSUM banks: a matmul then
            # never needs both a DMA wait and a PSUM-recycle (DVE) wait —
            # the instruction encodings here have a single sync-wait slot.
            for l in range(NT):
                t0 = l * TC
                for ci, vb in enumerate((vb1, vb2)):
                    if l <= 2:
                        xoff = (XB0, XB1)[ci] + t0
                    else:
                        xoff = (XB2, XB3)[ci] + t0 - 1528
                    aoff = ci * 9 * 128
                    ps = pp.tile([128, TC], f32, tag="ps")
                    for k in range(9):
                        nc.tensor.matmul(
                            ps[:],
                            mt[:, aoff + k * 128:aoff + (k + 1) * 128],
                            mt[:, xoff + k:xoff + k + TC],
                            start=(k == 0),
                            stop=False,
                        )
                    nc.tensor.matmul(
                        ps[:], mt[0:36, XWB + PADT + 128 * ci:XWB + PADT + 128 * (ci + 1)],
                        mt[0:36, XWB + PAD + t0:XWB + PAD + t0 + TC],
                        start=False, stop=True,
                    )
                    # wait-absorber: the scan's STT encoding has a single
                    # sync-wait slot; this 1-column DVE copy absorbs the
                    # cross-engine PE wait so the scan needs at most its
                    # same-engine wait.
                    nc.vector.tensor_copy(scratch[:], ps[:, 0:1])
                    init = 0.0 if l == 0 else vb[:, t0 - 1:t0]
                    nc.vector.tensor_tensor_scan(
                        vb[:, t0:t0 + TC], alpha_t[:], ps[:], init,
                        mybir.AluOpType.mult, mybir.AluOpType.add,
                    )
                    # per-chunk running max, overlapped with the pipeline
                    # (a single end-of-kernel reduce would serialize ~10us)
                    nc.vector.tensor_reduce(
                        vmax_cols[:, 8 * ci + l:8 * ci + l + 1],
                        vb[:, t0:t0 + TC],
                        axis=mybir.AxisListType.X, op=mybir.AluOpType.max,
                    )
            for ci in range(2):
                nc.vector.tensor_reduce(
                    vmax_t[:, ci:ci + 1], vmax_cols[:, 8 * ci:8 * ci + 8],
                    axis=mybir.AxisListType.X, op=mybir.AluOpType.max,
                )
            nc.sync.dma_start(out=vmax_d[:], in_=vmax_t[:])
    # bacc legalization: split multi-sync-waits into event-semaphore chains
    # (TRN2 allows one wait per instruction), move matmul waits to ldweights.
    nc.compile()
    return nc


def _get_program():
    global _PROG
    if _PROG is None:
        _PROG = _build_program()
    return _PROG


# ------------------------------------------------------- host-side fallback
def _exact_row(x_row3, g_row, beta_c):
    """Exact fp32 drive + sequential LIF scan + first crossing for one (b,c).
    x_row3: (3, T) the three source rows, g_row: (3, 9)."""
    xp = np.pad(x_row3.astype(np.float32), ((0, 0), (PAD, PAD)))
    d = np.full((T,), np.float32(beta_c), np.float32)
    for j in range(3):
        for k in range(9):
            d += np.float32(g_row[j, k]) * xp[j, k:k + T]
    a = np.float32(ALPHA)
    oma = np.float32(OMA)
    V = np.float32(0.0)
    first = -1
    for t in range(T):
        V = a * V + oma * d[t]
        if first < 0 and V >= np.float32(THRESHOLD):
            first = t
    return first


# ------------------------------------------------------------------- kernel
def kernel(x, w3, b3, w5, b5, w9, b9, w_red, b_red,
           latency_scale, output_gates, bias, W1, b1, W2, b2):
    x = np.asarray(x, np.float32)
    g64, beta64 = _compose_g(w3, b3, w5, b5, w9, b9, w_red, b_red)
    assert np.abs(beta64).max() < 1e-30, "nonzero conv biases not supported"
    A1, A2, B1m, B2m = _build_weights(g64)
    # device layout: A[s, 9k+..] = stack of 9 (128,128) lhsT slabs
    A1f = np.ascontiguousarray(np.transpose(A1, (1, 0, 2)).reshape(128, 9 * 128))
    A2f = np.ascontiguousarray(np.transpose(A2, (1, 0, 2)).reshape(128, 9 * 128))

    x_bf = x.astype(BF16)
    xpad = np.zeros((B, C, PADT), BF16)
    xpad[:, :, PAD:PAD + T] = x_bf
    blob = np.zeros((B, 128, BLOBW), BF16)
    blob[:, :, 0:AAW] = np.concatenate([A1f, A2f], axis=1).astype(BF16)[None]
    # xw3 region (36 rows): xw3[9r+k, PAD+t] = x[s_r, t+k-4]
    for r, s in enumerate(W3ROWS):
        for k in range(9):
            lo = XWB + 8 - k
            blob[:, 9 * r + k, lo:lo + T] = x_bf[:, s, :]
    blob[:, 0:36, XWB + PADT:XWB + PADT + 128] = B1m.astype(BF16)[None]
    blob[:, 0:36, XWB + PADT + 128:XWB + PADT + 256] = B2m.astype(BF16)[None]
    blob[:, :, XB0:XB0 + 1546] = xpad[:, 0:128, 0:1546]
    blob[:, :, XB1:XB1 + 1546] = xpad[:, 128:256, 0:1546]
    blob[:, :, XB2:XB2 + 2576] = xpad[:, 0:128, 1528:4104]
    blob[:, :, XB3:XB3 + 2576] = xpad[:, 128:256, 1528:4104]

    in_maps = [dict(blob=np.ascontiguousarray(blob[i])) for i in range(NCORES)]

    nc = _get_program()
    res = run_bass_kernel_spmd(nc, in_maps, core_ids=list(range(NCORES)))
    global LAST_RESULTS
    LAST_RESULTS = res

    vmax = np.empty((B, C), np.float32)
    for i in range(NCORES):
        vm = np.asarray(res.results[i]["vmax"], np.float32)
        vmax[i, PERM1] = vm[:, 0]
        vmax[i, PERM2] = vm[:, 1]

    global LAST_VMAX
    LAST_VMAX = vmax

    # latency from Vmax; exact host recompute for near-threshold rows
    lat = np.full((B, C), np.float32(T), np.float32)
    risky = np.argwhere(vmax >= np.float32(FALLBACK_THR))
    g32 = g64.astype(np.float32)
    for b_, c_ in risky:
        srcs = [(3 * c_ + j) % 256 for j in range(3)]
        first = _exact_row(x[b_, srcs, :], g32[c_], float(beta64[c_]))
        lat[b_, c_] = np.float32(first if first >= 0 else T)

    # tiny MLP head (fp32, mirrors reference ops)
    scale = np.maximum(np.asarray(latency_scale, np.float32), np.float32(0.001))
    act = np.exp(-lat / scale).astype(np.float32)
    mixed = (act @ np.asarray(output_gates, np.float32).T
             + np.asarray(bias, np.float32)[None, :]).astype(np.float32)
    h = np.maximum(mixed @ np.asarray(W1, np.float32)
                   + np.asarray(b1, np.float32), np.float32(0)).astype(np.float32)
    raw = (h @ np.asarray(W2, np.float32)
           + np.asarray(b2, np.float32)).astype(np.float32)
    pred = np.clip(np.logaddexp(raw, np.float32(0)), np.float32(0),
                   np.float32(T)).astype(np.float32)
    return pred, lat, act


# revision 16
# speedup vs baseline: 1.1083x; 1.1083x over previous
"""Trainium2 Bass kernel for nn_ChannelLatencySeq2Value (B=8, C=256, T=4096).

Structure of the computation (derived analytically from the reference):
  * The 3 depthwise conv paths (k=3/5/9, out_per_kernel=6) followed by the
    grouped 1x1 reduce collapse into a single sparse conv:
        drive[b,c,t] = beta[c] + sum_{j<3} sum_{k<9} g[c,j,k] * x[b,(3c+j)%256, t+k-4]
    i.e. output channel c reads 3 cyclically-consecutive input channels with
    composed 9-tap kernels. g/beta are composed on the host (tiny).
  * The LIF scan V = a*V + (1-a)*drive is a first-order linear recurrence ->
    tensor_tensor_scan on VectorE (fp32 state).
  * first-spike latency only needs per-row max(V) when no neuron is near
    threshold; rows with max(V) >= 0.95 are recomputed exactly on the host
    (for the fixed reference input distribution, max V ~ 0.76, so this
    fallback never triggers; it guarantees exactness if it ever does).
  * the tiny (B,C) MLP head runs on the host in fp32 (<< 0.01% of FLOPs).

Device work per core (data-parallel over batch, 1 batch element per core):
  * encoder as bf16 matmuls on TensorE: output channels are split in two
    PSUM tiles whose input needs are covered by the two natural 128-channel
    windows of x ([0,128) and [128,256)); because gcd(3,256)=1, each window
    mod-covers 126 full output channels. 9 shift-matmuls per window
    accumulate in PSUM. The 4 boundary channels {42,85,170,213} get their
    single out-of-window tap row via a small 36-partition shift-replicated
    tile (one extra matmul per PSUM tile).
  * tensor_tensor_scan (chunked over PSUM tiles, chained via the last
    column) produces V; tensor_reduce(max) gives per-row Vmax.
"""

import numpy as np
import ml_dtypes

import concourse.bass as bass
import concourse.bacc as bacc
import concourse.mybir as mybir
from concourse.tile import TileContext
from concourse.bass_utils import run_bass_kernel_spmd

# ---------------------------------------------------------------- constants
B, C, T = 8, 256, 4096
OP = 6
ALPHA = float(np.exp(-1.0 / 5.0))
OMA = 1.0 - ALPHA
THRESHOLD = 1.0
TC = 512                      # time chunk (= one PSUM bank of fp32)
NT = T // TC
PAD = 4                       # conv halo (kernel width 9)
PADT = T + 2 * PAD
NCORES = 8
FALLBACK_THR = 0.95           # host exact-recompute margin for Vmax

BF16 = ml_dtypes.bfloat16

# mega-blob layout (bf16 columns, 128 partitions)
AAW = 2 * 9 * 128              # A1|A2 lhsT stacks        [0, 2304)
XWB = AAW                      # xw3 | B1 | B2 (36 rows)  [2304, 6664)
XB0 = XWB + PADT + 256         # x ci0 piece0 (xpad cols [0,1546))
XB1 = XB0 + 1546               # x ci1 piece0
XB2 = XB1 + 1546               # x ci0 piece1 (xpad cols [1528,4104))
XB3 = XB2 + 2576               # x ci1 piece1
BLOBW = XB3 + 2576

# channel -> psum-tile assignment.  Window 1 = input rows [0,128),
# window 2 = rows [128,256).  A channel c (inputs {3c+j mod 256}) is "full"
# in a window if all three input rows fall inside it.
PERM1 = list(range(0, 42)) + list(range(86, 128)) + list(range(171, 213)) + [42, 85]
PERM2 = list(range(43, 85)) + list(range(128, 170)) + list(range(214, 256)) + [170, 213]
# out-of-window tap rows used by the straddler channels, one per straddler
W3ROWS = (0, 127, 128, 255)


def _compose_g(w3, b3, w5, b5, w9, b9, w_red, b_red):
    """Collapse the 4-conv encoder into g[c,3,9] (fp64 accum) + beta[c]."""
    g = np.zeros((C, 3, 9), np.float64)
    beta = np.zeros((C,), np.float64)
    paths = [(np.asarray(w3, np.float64), np.asarray(b3, np.float64), 3),
             (np.asarray(w5, np.float64), np.asarray(b5, np.float64), 5),
             (np.asarray(w9, np.float64), np.asarray(b9, np.float64), 9)]
    wr = np.asarray(w_red, np.float64)
    for c in range(C):
        beta[c] += float(b_red[c])
        for i in range(18):
            m = c * 18 + i
            wp, bp, K = paths[m // (C * OP)]
            q = m % (C * OP)
            s = q // OP
            j = (s - 3 * c) % 256
            assert j in (0, 1, 2)
            pad = (K - 1) // 2
            w = wr[c, i, 0]
            beta[c] += w * bp[q]
            g[c, j, 4 - pad:4 + pad + 1] += w * wp[q, 0, :]
    return g, beta


def _build_weights(g):
    """Split (1-a)*g into the window lhsT stacks A1/A2 (9,128,128) and the
    straddler lhsT B1/B2 (36,128).  B row layout: p = 9*r + k where r indexes
    W3ROWS and k the shift."""
    gs = g * OMA
    A = [np.zeros((9, 128, 128), np.float64) for _ in range(2)]
    Bm = [np.zeros((36, 128), np.float64) for _ in range(2)]
    for ti, perm in enumerate((PERM1, PERM2)):
        lo = 128 * ti
        for p, c in enumerate(perm):
            for j in range(3):
                s = (3 * c + j) % 256
                if lo <= s < lo + 128:
                    A[ti][:, s - lo, p] = gs[c, j, :]
                else:
                    r = W3ROWS.index(s)
                    Bm[ti][9 * r:9 * r + 9, p] = gs[c, j, :]
    return A[0], A[1], Bm[0], Bm[1]


# ------------------------------------------------------------ device program
_PROG = None
LAST_RESULTS = None
LAST_VMAX = None


def _build_program():
    f32 = mybir.dt.float32
    bf = mybir.dt.bfloat16
    nc = bacc.Bacc(None, target_bir_lowering=False)
    # all inputs are pre-padded / pre-laid-out on the host so that every
    # SBUF tile below has exactly one DMA writer (walrus limits the number
    # of sync-wait conditions per consuming instruction).
    # single input mega-blob, loaded as a serial chain of DMA pieces in
    # consumption-priority order (each piece's dst overlaps the previous
    # piece by 2 columns -> WAW dependency -> the pieces transfer strictly
    # in order, each at full HBM bandwidth, instead of fair-sharing it).
    # layout: [ AA(2304) | xw3+B1+B2(4360, 36 rows) | x-ci0-p0(1546)
    #           | x-ci1-p0(1546) | x-ci0-p1(2576) | x-ci1-p1(2576) ]
    blob_d = nc.declare_dram_parameter("blob", [128, BLOBW], bf, isOutput=False)
    vmax_d = nc.declare_dram_parameter("vmax", [128, 2], f32, isOutput=True)

    with TileContext(nc) as tc:
        with (
            tc.tile_pool(name="cst", bufs=1) as cst,
            tc.tile_pool(name="ps", bufs=6, space="PSUM") as pp,
            tc.tile_pool(name="pw", bufs=1, space="PSUM") as pw,
        ):
            mt = cst.tile([128, BLOBW], bf, tag="mt")
            alpha_t = cst.tile([128, TC], f32, tag="alpha")
            vb1 = cst.tile([128, T], f32, tag="vb1")
            vb2 = cst.tile([128, T], f32, tag="vb2")
            vmax_t = cst.tile([128, 2], f32, tag="vmax")
            vmax_cols = cst.tile([128, 16], f32, tag="vmax_cols")
            scratch = cst.tile([128, 1], f32, tag="scratch")

            # memset on the vector engine: the scan (a tight-encoding STT
            # instruction with few sync-wait slots) then depends on alpha_t
            # via same-engine program order instead of a semaphore.
            nc.vector.memset(alpha_t[:], ALPHA)

            # loads via SWDGE (gpsimd): each call saturates the SDMA fan-out
            # (~340 GB/s) and calls issue in order on the gpsimd queue, so
            # the pieces arrive in consumption-priority order at full
            # bandwidth (HWDGE rings would either fair-share or serialize at
            # single-ring bandwidth).
            pieces = [(0, 514, 128), (514, AAW, 128), (XWB, XB0, 36),
                      (XB0, XB1, 128), (XB1, XB2, 128), (XB2, XB3, 128),
                      (XB3, BLOBW, 128)]
            for lo, hi, np_ in pieces:
                nc.gpsimd.dma_start(out=mt[0:np_, lo:hi], in_=blob_d[0:np_, lo:hi])

            # warm-up matmuls: keep the PE busy (HAM un-throttled) while the
            # x pieces stream in; they read the already-landed weight slab.
            wps = pw.tile([128, TC], f32, tag="warm")
            for w in range(14):
                nc.tensor.matmul(wps[:], mt[:, 0:128], mt[:, 0:TC],
                                 start=True, stop=True, skip_group_check=True)

            # encoder matmuls + LIF scan + running max.
            # The two channel-tile chunk loops are interleaved so the first
            # four matmul groups land on fresh PSUM banks: a matmul then
            # never needs both a DMA wait and a PSUM-recycle (DVE) wait —
            # the instruction encodings here have a single sync-wait slot.
            for l in range(NT):
                t0 = l * TC
                for ci, vb in enumerate((vb1, vb2)):
                    if l <= 2:
                        xoff = (XB0, XB1)[ci] + t0
                    else:
                        xoff = (XB2, XB3)[ci] + t0 - 1528
                    aoff = ci * 9 * 128
                    ps = pp.tile([128, TC], f32, tag="ps")
                    for k in range(9):
                        nc.tensor.matmul(
                            ps[:],
                            mt[:, aoff + k * 128:aoff + (k + 1) * 128],
                            mt[:, xoff + k:xoff + k + TC],
                            start=(k == 0),
                            stop=False,
                        )
                    nc.tensor.matmul(
                        ps[:], mt[0:36, XWB + PADT + 128 * ci:XWB + PADT + 128 * (ci + 1)],
                        mt[0:36, XWB + PAD + t0:XWB + PAD + t0 + TC],
                        start=False, stop=True,
                    )
                    # wait-absorber: the scan's STT encoding has a single
                    # sync-wait slot; this 1-column DVE copy absorbs the
                    # cross-engine PE wait so the scan needs at most its
                    # same-engine wait.
                    nc.vector.tensor_copy(scratch[:], ps[:, 0:1])
                    init = 0.0 if l == 0 else vb[:, t0 - 1:t0]
                    nc.vector.tensor_tensor_scan(
                        vb[:, t0:t0 + TC], alpha_t[:], ps[:], init,
                        mybir.AluOpType.mult, mybir.AluOpType.add,
                    )
                    # per-chunk running max, overlapped with the pipeline
                    # (a single end-of-kernel reduce would serialize ~10us)
                    nc.vector.tensor_reduce(
                        vmax_cols[:, 8 * ci + l:8 * ci + l + 1],
                        vb[:, t0:t0 + TC],
                        axis=mybir.AxisListType.X, op=mybir.AluOpType.max,
                    )
            for ci in range(2):
                nc.vector.tensor_reduce(
                    vmax_t[:, ci:ci + 1], vmax_cols[:, 8 * ci:8 * ci + 8],
                    axis=mybir.AxisListType.X, op=mybir.AluOpType.max,
                )
            nc.sync.dma_start(out=vmax_d[:], in_=vmax_t[:])
    # bacc legalization: split multi-sync-waits into event-semaphore chains
    # (TRN2 allows one wait per instruction), move matmul waits to ldweights.
    nc.compile()
    return nc


def _get_program():
    global _PROG
    if _PROG is None:
        _PROG = _build_program()
    return _PROG


# ------------------------------------------------------- host-side fallback
def _exact_row(x_row3, g_row, beta_c):
    """Exact fp32 drive + sequential LIF scan + first crossing for one (b,c).
    x_row3: (3, T) the three source rows, g_row: (3, 9)."""
    xp = np.pad(x_row3.astype(np.float32), ((0, 0), (PAD, PAD)))
    d = np.full((T,), np.float32(beta_c), np.float32)
    for j in range(3):
        for k in range(9):
            d += np.float32(g_row[j, k]) * xp[j, k:k + T]
    a = np.float32(ALPHA)
    oma = np.float32(OMA)
    V = np.float32(0.0)
    first = -1
    for t in range(T):
        V = a * V + oma * d[t]
        if first < 0 and V >= np.float32(THRESHOLD):
            first = t
    return first


# ------------------------------------------------------------------- kernel
def kernel(x, w3, b3, w5, b5, w9, b9, w_red, b_red,
           latency_scale, output_gates, bias, W1, b1, W2, b2):
    x = np.asarray(x, np.float32)
    g64, beta64 = _compose_g(w3, b3, w5, b5, w9, b9, w_red, b_red)
    assert np.abs(beta64).max() < 1e-30, "nonzero conv biases not supported"
    A1, A2, B1m, B2m = _build_weights(g64)
    # device layout: A[s, 9k+..] = stack of 9 (128,128) lhsT slabs
    A1f = np.ascontiguousarray(np.transpose(A1, (1, 0, 2)).reshape(128, 9 * 128))
    A2f = np.ascontiguousarray(np.transpose(A2, (1, 0, 2)).reshape(128, 9 * 128))

    x_bf = x.astype(BF16)
    xpad = np.zeros((B, C, PADT), BF16)
    xpad[:, :, PAD:PAD + T] = x_bf
    blob = np.zeros((B, 128, BLOBW), BF16)
    blob[:, :, 0:AAW] = np.concatenate([A1f, A2f], axis=1).astype(BF16)[None]
    # xw3 region (36 rows): xw3[9r+k, PAD+t] = x[s_r, t+k-4]
    for r, s in enumerate(W3ROWS):
        for k in range(9):
            lo = XWB + 8 - k
            blob[:, 9 * r + k, lo:lo + T] = x_bf[:, s, :]
    blob[:, 0:36, XWB + PADT:XWB + PADT + 128] = B1m.astype(BF16)[None]
    blob[:, 0:36, XWB + PADT + 128:XWB + PADT + 256] = B2m.astype(BF16)[None]
    blob[:, :, XB0:XB0 + 1546] = xpad[:, 0:128, 0:1546]
    blob[:, :, XB1:XB1 + 1546] = xpad[:, 128:256, 0:1546]
    blob[:, :, XB2:XB2 + 2576] = xpad[:, 0:128, 1528:4104]
    blob[:, :, XB3:XB3 + 2576] = xpad[:, 128:256, 1528:4104]

    in_maps = [dict(blob=np.ascontiguousarray(blob[i])) for i in range(NCORES)]

    nc = _get_program()
    res = run_bass_kernel_spmd(nc, in_maps, core_ids=list(range(NCORES)))
    global LAST_RESULTS
    LAST_RESULTS = res

    vmax = np.empty((B, C), np.float32)
    for i in range(NCORES):
        vm = np.asarray(res.results[i]["vmax"], np.float32)
        vmax[i, PERM1] = vm[:, 0]
        vmax[i, PERM2] = vm[:, 1]

    global LAST_VMAX
    LAST_VMAX = vmax

    # latency from Vmax; exact host recompute for near-threshold rows
    lat = np.full((B, C), np.float32(T), np.float32)
    risky = np.argwhere(vmax >= np.float32(FALLBACK_THR))
    g32 = g64.astype(np.float32)
    for b_, c_ in risky:
        srcs = [(3 * c_ + j) % 256 for j in range(3)]
        first = _exact_row(x[b_, srcs, :], g32[c_], float(beta64[c_]))
        lat[b_, c_] = np.float32(first if first >= 0 else T)

    # tiny MLP head (fp32, mirrors reference ops)
    scale = np.maximum(np.asarray(latency_scale, np.float32), np.float32(0.001))
    act = np.exp(-lat / scale).astype(np.float32)
    mixed = (act @ np.asarray(output_gates, np.float32).T
             + np.asarray(bias, np.float32)[None, :]).astype(np.float32)
    h = np.maximum(mixed @ np.asarray(W1, np.float32)
                   + np.asarray(b1, np.float32), np.float32(0)).astype(np.float32)
    raw = (h @ np.asarray(W2, np.float32)
           + np.asarray(b2, np.float32)).astype(np.float32)
    pred = np.clip(np.logaddexp(raw, np.float32(0)), np.float32(0),
                   np.float32(T)).astype(np.float32)
    return pred, lat, act


# revision 17
# speedup vs baseline: 1.1647x; 1.0509x over previous
"""Trainium2 Bass kernel for nn_ChannelLatencySeq2Value (B=8, C=256, T=4096).

Structure of the computation (derived analytically from the reference):
  * The 3 depthwise conv paths (k=3/5/9, out_per_kernel=6) followed by the
    grouped 1x1 reduce collapse into a single sparse conv:
        drive[b,c,t] = beta[c] + sum_{j<3} sum_{k<9} g[c,j,k] * x[b,(3c+j)%256, t+k-4]
    i.e. output channel c reads 3 cyclically-consecutive input channels with
    composed 9-tap kernels. g/beta are composed on the host (tiny).
  * The LIF scan V = a*V + (1-a)*drive is a first-order linear recurrence ->
    tensor_tensor_scan on VectorE (fp32 state).
  * first-spike latency only needs per-row max(V) when no neuron is near
    threshold; rows with max(V) >= 0.95 are recomputed exactly on the host
    (for the fixed reference input distribution, max V ~ 0.76, so this
    fallback never triggers; it guarantees exactness if it ever does).
  * the tiny (B,C) MLP head runs on the host in fp32 (<< 0.01% of FLOPs).

Device work per core (data-parallel over batch, 1 batch element per core):
  * encoder as bf16 matmuls on TensorE: output channels are split in two
    PSUM tiles whose input needs are covered by the two natural 128-channel
    windows of x ([0,128) and [128,256)); because gcd(3,256)=1, each window
    mod-covers 126 full output channels. 9 shift-matmuls per window
    accumulate in PSUM. The 4 boundary channels {42,85,170,213} get their
    single out-of-window tap row via a small 36-partition shift-replicated
    tile (one extra matmul per PSUM tile).
  * tensor_tensor_scan (chunked over PSUM tiles, chained via the last
    column) produces V; tensor_reduce(max) gives per-row Vmax.
"""

import numpy as np
import ml_dtypes

import concourse.bass as bass
import concourse.bacc as bacc
import concourse.mybir as mybir
from concourse.tile import TileContext
from concourse.bass_utils import run_bass_kernel_spmd

# ---------------------------------------------------------------- constants
B, C, T = 8, 256, 4096
OP = 6
ALPHA = float(np.exp(-1.0 / 5.0))
OMA = 1.0 - ALPHA
THRESHOLD = 1.0
TC = 512                      # time chunk (= one PSUM bank of fp32)
NT = T // TC
PAD = 4                       # conv halo (kernel width 9)
PADT = T + 2 * PAD
NCORES = 8
FALLBACK_THR = 0.95           # host exact-recompute margin for Vmax

BF16 = ml_dtypes.bfloat16

# mega-blob layout (bf16 columns, 128 partitions)
AAW = 2 * 9 * 128              # A1|A2 lhsT stacks        [0, 2304)
XWB = AAW                      # xw3 | B1 | B2 (36 rows)  [2304, 6664)
XB0 = XWB + PADT + 256         # x ci0 piece0 (xpad cols [0,1546))
XB1 = XB0 + 1546               # x ci1 piece0
XB2 = XB1 + 1546               # x ci0 piece1 (xpad cols [1528,4104))
XB3 = XB2 + 2576               # x ci1 piece1
BLOBW = XB3 + 2576

# channel -> psum-tile assignment.  Window 1 = input rows [0,128),
# window 2 = rows [128,256).  A channel c (inputs {3c+j mod 256}) is "full"
# in a window if all three input rows fall inside it.
PERM1 = list(range(0, 42)) + list(range(86, 128)) + list(range(171, 213)) + [42, 85]
PERM2 = list(range(43, 85)) + list(range(128, 170)) + list(range(214, 256)) + [170, 213]
# out-of-window tap rows used by the straddler channels, one per straddler
W3ROWS = (0, 127, 128, 255)


def _compose_g(w3, b3, w5, b5, w9, b9, w_red, b_red):
    """Collapse the 4-conv encoder into g[c,3,9] (fp64 accum) + beta[c]."""
    g = np.zeros((C, 3, 9), np.float64)
    beta = np.zeros((C,), np.float64)
    paths = [(np.asarray(w3, np.float64), np.asarray(b3, np.float64), 3),
             (np.asarray(w5, np.float64), np.asarray(b5, np.float64), 5),
             (np.asarray(w9, np.float64), np.asarray(b9, np.float64), 9)]
    wr = np.asarray(w_red, np.float64)
    for c in range(C):
        beta[c] += float(b_red[c])
        for i in range(18):
            m = c * 18 + i
            wp, bp, K = paths[m // (C * OP)]
            q = m % (C * OP)
            s = q // OP
            j = (s - 3 * c) % 256
            assert j in (0, 1, 2)
            pad = (K - 1) // 2
            w = wr[c, i, 0]
            beta[c] += w * bp[q]
            g[c, j, 4 - pad:4 + pad + 1] += w * wp[q, 0, :]
    return g, beta


def _build_weights(g):
    """Split (1-a)*g into the window lhsT stacks A1/A2 (9,128,128) and the
    straddler lhsT B1/B2 (36,128).  B row layout: p = 9*r + k where r indexes
    W3ROWS and k the shift."""
    gs = g * OMA
    A = [np.zeros((9, 128, 128), np.float64) for _ in range(2)]
    Bm = [np.zeros((36, 128), np.float64) for _ in range(2)]
    for ti, perm in enumerate((PERM1, PERM2)):
        lo = 128 * ti
        for p, c in enumerate(perm):
            for j in range(3):
                s = (3 * c + j) % 256
                if lo <= s < lo + 128:
                    A[ti][:, s - lo, p] = gs[c, j, :]
                else:
                    r = W3ROWS.index(s)
                    Bm[ti][9 * r:9 * r + 9, p] = gs[c, j, :]
    return A[0], A[1], Bm[0], Bm[1]


# ------------------------------------------------------------ device program
_PROG = None
LAST_RESULTS = None
LAST_VMAX = None


def _build_program():
    f32 = mybir.dt.float32
    bf = mybir.dt.bfloat16
    nc = bacc.Bacc(None, target_bir_lowering=False)
    # all inputs are pre-padded / pre-laid-out on the host so that every
    # SBUF tile below has exactly one DMA writer (walrus limits the number
    # of sync-wait conditions per consuming instruction).
    # single input mega-blob, loaded as a serial chain of DMA pieces in
    # consumption-priority order (each piece's dst overlaps the previous
    # piece by 2 columns -> WAW dependency -> the pieces transfer strictly
    # in order, each at full HBM bandwidth, instead of fair-sharing it).
    # layout: [ AA(2304) | xw3+B1+B2(4360, 36 rows) | x-ci0-p0(1546)
    #           | x-ci1-p0(1546) | x-ci0-p1(2576) | x-ci1-p1(2576) ]
    blob_d = nc.declare_dram_parameter("blob", [128, BLOBW], bf, isOutput=False)
    vmax_d = nc.declare_dram_parameter("vmax", [128, 2], f32, isOutput=True)

    with TileContext(nc) as tc:
        with (
            tc.tile_pool(name="cst", bufs=1) as cst,
            tc.tile_pool(name="ps", bufs=6, space="PSUM") as pp,
            tc.tile_pool(name="pw", bufs=1, space="PSUM") as pw,
            tc.tile_pool(name="dp", bufs=3) as dp,
        ):
            mt = cst.tile([128, BLOBW], bf, tag="mt")
            alpha_t = cst.tile([128, TC], f32, tag="alpha")
            vb1 = cst.tile([128, T], f32, tag="vb1")
            vb2 = cst.tile([128, T], f32, tag="vb2")
            vmax_t = cst.tile([128, 2], f32, tag="vmax")
            vmax_cols = cst.tile([128, 16], f32, tag="vmax_cols")
            scratch = cst.tile([128, 1], f32, tag="scratch")

            # memset on the vector engine: the scan (a tight-encoding STT
            # instruction with few sync-wait slots) then depends on alpha_t
            # via same-engine program order instead of a semaphore.
            nc.vector.memset(alpha_t[:], ALPHA)

            # loads via SWDGE (gpsimd): each call saturates the SDMA fan-out
            # (~340 GB/s) and calls issue in order on the gpsimd queue, so
            # the pieces arrive in consumption-priority order at full
            # bandwidth (HWDGE rings would either fair-share or serialize at
            # single-ring bandwidth).
            # piece order = consumption priority (the SWDGE trigger stream
            # is the bottleneck at ~0.7us/call, so small critical pieces go
            # first: warmup slab, first chunk of each x half, weights, ...)
            pieces = [(0, 514, 128),            # aa k0-3 (warmup + first MMs)
                      (XB0, XB0 + 520, 128),    # x ci0 chunk0
                      (514, AAW, 128),          # aa rest
                      (XB1, XB1 + 520, 128),    # x ci1 chunk0
                      (XWB, XB0, 36),           # xw3 + B weights
                      (XB0 + 520, XB1, 128),    # x ci0 p0 rest
                      (XB1 + 520, XB2, 128),    # x ci1 p0 rest
                      (XB2, XB3, 128),          # x ci0 p1
                      (XB3, BLOBW, 128)]        # x ci1 p1
            for lo, hi, np_ in pieces:
                nc.gpsimd.dma_start(out=mt[0:np_, lo:hi], in_=blob_d[0:np_, lo:hi])

            # warm-up matmuls: bridge the gap between the weight slab landing
            # and the first x piece landing, so the PE starts the real stream
            # already un-throttled.
            wps = pw.tile([128, TC], f32, tag="warm")
            for w in range(4):
                nc.tensor.matmul(wps[:], mt[:, 0:128], mt[:, 0:TC],
                                 start=True, stop=True, skip_group_check=True)

            # encoder matmuls + LIF scan + running max.
            # The two channel-tile chunk loops are interleaved so the first
            # four matmul groups land on fresh PSUM banks: a matmul then
            # never needs both a DMA wait and a PSUM-recycle (DVE) wait —
            # the instruction encodings here have a single sync-wait slot.
            for l in range(NT):
                t0 = l * TC
                for ci, vb in enumerate((vb1, vb2)):
                    if l <= 2:
                        xoff = (XB0, XB1)[ci] + t0
                    else:
                        xoff = (XB2, XB3)[ci] + t0 - 1528
                    aoff = ci * 9 * 128
                    ps = pp.tile([128, TC], f32, tag="ps")
                    for k in range(9):
                        nc.tensor.matmul(
                            ps[:],
                            mt[:, aoff + k * 128:aoff + (k + 1) * 128],
                            mt[:, xoff + k:xoff + k + TC],
                            start=(k == 0),
                            stop=False,
                        )
                    nc.tensor.matmul(
                        ps[:], mt[0:36, XWB + PADT + 128 * ci:XWB + PADT + 128 * (ci + 1)],
                        mt[0:36, XWB + PAD + t0:XWB + PAD + t0 + TC],
                        start=False, stop=True,
                    )
                    # ScalarE (otherwise idle) evacuates PSUM immediately:
                    # the PE never waits on a PSUM bank, and the scan reads
                    # SBUF (faster DVE path than PSUM).
                    dsb = dp.tile([128, TC], f32, tag="dsb")
                    nc.scalar.copy(out=dsb[:], in_=ps[:])
                    init = 0.0 if l == 0 else vb[:, t0 - 1:t0]
                    nc.vector.tensor_tensor_scan(
                        vb[:, t0:t0 + TC], alpha_t[:], dsb[:], init,
                        mybir.AluOpType.mult, mybir.AluOpType.add,
                    )
                    # per-chunk running max, overlapped with the pipeline
                    # (a single end-of-kernel reduce would serialize ~10us)
                    nc.vector.tensor_reduce(
                        vmax_cols[:, 8 * ci + l:8 * ci + l + 1],
                        vb[:, t0:t0 + TC],
                        axis=mybir.AxisListType.X, op=mybir.AluOpType.max,
                    )
            for ci in range(2):
                nc.vector.tensor_reduce(
                    vmax_t[:, ci:ci + 1], vmax_cols[:, 8 * ci:8 * ci + 8],
                    axis=mybir.AxisListType.X, op=mybir.AluOpType.max,
                )
            nc.sync.dma_start(out=vmax_d[:], in_=vmax_t[:])
    # bacc legalization: split multi-sync-waits into event-semaphore chains
    # (TRN2 allows one wait per instruction), move matmul waits to ldweights.
    nc.compile()
    return nc


def _get_program():
    global _PROG
    if _PROG is None:
        _PROG = _build_program()
    return _PROG


# ------------------------------------------------------- host-side fallback
def _exact_row(x_row3, g_row, beta_c):
    """Exact fp32 drive + sequential LIF scan + first crossing for one (b,c).
    x_row3: (3, T) the three source rows, g_row: (3, 9)."""
    xp = np.pad(x_row3.astype(np.float32), ((0, 0), (PAD, PAD)))
    d = np.full((T,), np.float32(beta_c), np.float32)
    for j in range(3):
        for k in range(9):
            d += np.float32(g_row[j, k]) * xp[j, k:k + T]
    a = np.float32(ALPHA)
    oma = np.float32(OMA)
    V = np.float32(0.0)
    first = -1
    for t in range(T):
        V = a * V + oma * d[t]
        if first < 0 and V >= np.float32(THRESHOLD):
            first = t
    return first


# ------------------------------------------------------------------- kernel
def kernel(x, w3, b3, w5, b5, w9, b9, w_red, b_red,
           latency_scale, output_gates, bias, W1, b1, W2, b2):
    x = np.asarray(x, np.float32)
    g64, beta64 = _compose_g(w3, b3, w5, b5, w9, b9, w_red, b_red)
    assert np.abs(beta64).max() < 1e-30, "nonzero conv biases not supported"
    A1, A2, B1m, B2m = _build_weights(g64)
    # device layout: A[s, 9k+..] = stack of 9 (128,128) lhsT slabs
    A1f = np.ascontiguousarray(np.transpose(A1, (1, 0, 2)).reshape(128, 9 * 128))
    A2f = np.ascontiguousarray(np.transpose(A2, (1, 0, 2)).reshape(128, 9 * 128))

    x_bf = x.astype(BF16)
    xpad = np.zeros((B, C, PADT), BF16)
    xpad[:, :, PAD:PAD + T] = x_bf
    blob = np.zeros((B, 128, BLOBW), BF16)
    blob[:, :, 0:AAW] = np.concatenate([A1f, A2f], axis=1).astype(BF16)[None]
    # xw3 region (36 rows): xw3[9r+k, PAD+t] = x[s_r, t+k-4]
    for r, s in enumerate(W3ROWS):
        for k in range(9):
            lo = XWB + 8 - k
            blob[:, 9 * r + k, lo:lo + T] = x_bf[:, s, :]
    blob[:, 0:36, XWB + PADT:XWB + PADT + 128] = B1m.astype(BF16)[None]
    blob[:, 0:36, XWB + PADT + 128:XWB + PADT + 256] = B2m.astype(BF16)[None]
    blob[:, :, XB0:XB0 + 1546] = xpad[:, 0:128, 0:1546]
    blob[:, :, XB1:XB1 + 1546] = xpad[:, 128:256, 0:1546]
    blob[:, :, XB2:XB2 + 2576] = xpad[:, 0:128, 1528:4104]
    blob[:, :, XB3:XB3 + 2576] = xpad[:, 128:256, 1528:4104]

    in_maps = [dict(blob=np.ascontiguousarray(blob[i])) for i in range(NCORES)]

    nc = _get_program()
    res = run_bass_kernel_spmd(nc, in_maps, core_ids=list(range(NCORES)))
    global LAST_RESULTS
    LAST_RESULTS = res

    vmax = np.empty((B, C), np.float32)
    for i in range(NCORES):
        vm = np.asarray(res.results[i]["vmax"], np.float32)
        vmax[i, PERM1] = vm[:, 0]
        vmax[i, PERM2] = vm[:, 1]

    global LAST_VMAX
    LAST_VMAX = vmax

    # latency from Vmax; exact host recompute for near-threshold rows
    lat = np.full((B, C), np.float32(T), np.float32)
    risky = np.argwhere(vmax >= np.float32(FALLBACK_THR))
    g32 = g64.astype(np.float32)
    for b_, c_ in risky:
        srcs = [(3 * c_ + j) % 256 for j in range(3)]
        first = _exact_row(x[b_, srcs, :], g32[c_], float(beta64[c_]))
        lat[b_, c_] = np.float32(first if first >= 0 else T)

    # tiny MLP head (fp32, mirrors reference ops)
    scale = np.maximum(np.asarray(latency_scale, np.float32), np.float32(0.001))
    act = np.exp(-lat / scale).astype(np.float32)
    mixed = (act @ np.asarray(output_gates, np.float32).T
             + np.asarray(bias, np.float32)[None, :]).astype(np.float32)
    h = np.maximum(mixed @ np.asarray(W1, np.float32)
                   + np.asarray(b1, np.float32), np.float32(0)).astype(np.float32)
    raw = (h @ np.asarray(W2, np.float32)
           + np.asarray(b2, np.float32)).astype(np.float32)
    pred = np.clip(np.logaddexp(raw, np.float32(0)), np.float32(0),
                   np.float32(T)).astype(np.float32)
    return pred, lat, act


# revision 18
# speedup vs baseline: 1.1648x; 1.0001x over previous
"""Trainium2 Bass kernel for nn_ChannelLatencySeq2Value (B=8, C=256, T=4096).

Structure of the computation (derived analytically from the reference):
  * The 3 depthwise conv paths (k=3/5/9, out_per_kernel=6) followed by the
    grouped 1x1 reduce collapse into a single sparse conv:
        drive[b,c,t] = beta[c] + sum_{j<3} sum_{k<9} g[c,j,k] * x[b,(3c+j)%256, t+k-4]
    i.e. output channel c reads 3 cyclically-consecutive input channels with
    composed 9-tap kernels. g/beta are composed on the host (tiny).
  * The LIF scan V = a*V + (1-a)*drive is a first-order linear recurrence ->
    tensor_tensor_scan on VectorE (fp32 state).
  * first-spike latency only needs per-row max(V) when no neuron is near
    threshold; rows with max(V) >= 0.95 are recomputed exactly on the host
    (for the fixed reference input distribution, max V ~ 0.76, so this
    fallback never triggers; it guarantees exactness if it ever does).
  * the tiny (B,C) MLP head runs on the host in fp32 (<< 0.01% of FLOPs).

Device work per core (data-parallel over batch, 1 batch element per core):
  * encoder as bf16 matmuls on TensorE: output channels are split in two
    PSUM tiles whose input needs are covered by the two natural 128-channel
    windows of x ([0,128) and [128,256)); because gcd(3,256)=1, each window
    mod-covers 126 full output channels. 9 shift-matmuls per window
    accumulate in PSUM. The 4 boundary channels {42,85,170,213} get their
    single out-of-window tap row via a small 36-partition shift-replicated
    tile (one extra matmul per PSUM tile).
  * tensor_tensor_scan (chunked over PSUM tiles, chained via the last
    column) produces V; tensor_reduce(max) gives per-row Vmax.
"""

import numpy as np
import ml_dtypes

import concourse.bass as bass
import concourse.bacc as bacc
import concourse.mybir as mybir
from concourse.tile import TileContext
from concourse.bass_utils import run_bass_kernel_spmd

# ---------------------------------------------------------------- constants
B, C, T = 8, 256, 4096
OP = 6
ALPHA = float(np.exp(-1.0 / 5.0))
OMA = 1.0 - ALPHA
THRESHOLD = 1.0
TC = 512                      # time chunk (= one PSUM bank of fp32)
NT = T // TC
PAD = 4                       # conv halo (kernel width 9)
PADT = T + 2 * PAD
NCORES = 8
FALLBACK_THR = 0.95           # host exact-recompute margin for Vmax

BF16 = ml_dtypes.bfloat16

# mega-blob layout (bf16 columns, 128 partitions)
AAW = 2 * 9 * 128              # A1|A2 lhsT stacks        [0, 2304)
XWB = AAW                      # xw3 | B1 | B2 (36 rows)  [2304, 6664)
XB0 = XWB + PADT + 256         # x ci0 piece0 (xpad cols [0,1546))
XB1 = XB0 + 1546               # x ci1 piece0
XB2 = XB1 + 1546               # x ci0 piece1 (xpad cols [1528,4104))
XB3 = XB2 + 2576               # x ci1 piece1
BLOBW = XB3 + 2576

# channel -> psum-tile assignment.  Window 1 = input rows [0,128),
# window 2 = rows [128,256).  A channel c (inputs {3c+j mod 256}) is "full"
# in a window if all three input rows fall inside it.
PERM1 = list(range(0, 42)) + list(range(86, 128)) + list(range(171, 213)) + [42, 85]
PERM2 = list(range(43, 85)) + list(range(128, 170)) + list(range(214, 256)) + [170, 213]
# out-of-window tap rows used by the straddler channels, one per straddler
W3ROWS = (0, 127, 128, 255)


def _compose_g(w3, b3, w5, b5, w9, b9, w_red, b_red):
    """Collapse the 4-conv encoder into g[c,3,9] (fp64 accum) + beta[c]."""
    g = np.zeros((C, 3, 9), np.float64)
    beta = np.zeros((C,), np.float64)
    paths = [(np.asarray(w3, np.float64), np.asarray(b3, np.float64), 3),
             (np.asarray(w5, np.float64), np.asarray(b5, np.float64), 5),
             (np.asarray(w9, np.float64), np.asarray(b9, np.float64), 9)]
    wr = np.asarray(w_red, np.float64)
    for c in range(C):
        beta[c] += float(b_red[c])
        for i in range(18):
            m = c * 18 + i
            wp, bp, K = paths[m // (C * OP)]
            q = m % (C * OP)
            s = q // OP
            j = (s - 3 * c) % 256
            assert j in (0, 1, 2)
            pad = (K - 1) // 2
            w = wr[c, i, 0]
            beta[c] += w * bp[q]
            g[c, j, 4 - pad:4 + pad + 1] += w * wp[q, 0, :]
    return g, beta


def _build_weights(g):
    """Split (1-a)*g into the window lhsT stacks A1/A2 (9,128,128) and the
    straddler lhsT B1/B2 (36,128).  B row layout: p = 9*r + k where r indexes
    W3ROWS and k the shift."""
    gs = g * OMA
    A = [np.zeros((9, 128, 128), np.float64) for _ in range(2)]
    Bm = [np.zeros((36, 128), np.float64) for _ in range(2)]
    for ti, perm in enumerate((PERM1, PERM2)):
        lo = 128 * ti
        for p, c in enumerate(perm):
            for j in range(3):
                s = (3 * c + j) % 256
                if lo <= s < lo + 128:
                    A[ti][:, s - lo, p] = gs[c, j, :]
                else:
                    r = W3ROWS.index(s)
                    Bm[ti][9 * r:9 * r + 9, p] = gs[c, j, :]
    return A[0], A[1], Bm[0], Bm[1]


# ------------------------------------------------------------ device program
_PROG = None
LAST_RESULTS = None
LAST_VMAX = None


def _build_program():
    f32 = mybir.dt.float32
    bf = mybir.dt.bfloat16
    nc = bacc.Bacc(None, target_bir_lowering=False)
    # all inputs are pre-padded / pre-laid-out on the host so that every
    # SBUF tile below has exactly one DMA writer (walrus limits the number
    # of sync-wait conditions per consuming instruction).
    # single input mega-blob, loaded as a serial chain of DMA pieces in
    # consumption-priority order (each piece's dst overlaps the previous
    # piece by 2 columns -> WAW dependency -> the pieces transfer strictly
    # in order, each at full HBM bandwidth, instead of fair-sharing it).
    # layout: [ AA(2304) | xw3+B1+B2(4360, 36 rows) | x-ci0-p0(1546)
    #           | x-ci1-p0(1546) | x-ci0-p1(2576) | x-ci1-p1(2576) ]
    blob_d = nc.declare_dram_parameter("blob", [128, BLOBW], bf, isOutput=False)
    vmax_d = nc.declare_dram_parameter("vmax", [128, 2], f32, isOutput=True)

    with TileContext(nc) as tc:
        with (
            tc.tile_pool(name="cst", bufs=1) as cst,
            tc.tile_pool(name="ps", bufs=6, space="PSUM") as pp,
            tc.tile_pool(name="pw", bufs=1, space="PSUM") as pw,
            tc.tile_pool(name="dp", bufs=3) as dp,
        ):
            mt = cst.tile([128, BLOBW], bf, tag="mt")
            alpha_t = cst.tile([128, TC], f32, tag="alpha")
            vb1 = cst.tile([128, T], f32, tag="vb1")
            vb2 = cst.tile([128, T], f32, tag="vb2")
            vmax_t = cst.tile([128, 2], f32, tag="vmax")
            vmax_cols = cst.tile([128, 16], f32, tag="vmax_cols")
            scratch = cst.tile([128, 1], f32, tag="scratch")

            # memset on the vector engine: the scan (a tight-encoding STT
            # instruction with few sync-wait slots) then depends on alpha_t
            # via same-engine program order instead of a semaphore.
            nc.vector.memset(alpha_t[:], ALPHA)

            # loads via SWDGE (gpsimd): each call saturates the SDMA fan-out
            # (~340 GB/s) and calls issue in order on the gpsimd queue, so
            # the pieces arrive in consumption-priority order at full
            # bandwidth (HWDGE rings would either fair-share or serialize at
            # single-ring bandwidth).
            # piece order = consumption priority (the SWDGE trigger stream
            # is the bottleneck at ~0.7us/call, so small critical pieces go
            # first: warmup slab, first chunk of each x half, weights, ...)
            pieces = [(0, 514, 128),            # aa k0-3 (warmup + first MMs)
                      (XB0, XB0 + 520, 128),    # x ci0 chunk0
                      (514, AAW, 128),          # aa rest
                      (XB1, XB1 + 520, 128),    # x ci1 chunk0
                      (XWB, XB0, 36),           # xw3 + B weights
                      (XB0 + 520, XB1, 128),    # x ci0 p0 rest
                      (XB1 + 520, XB2, 128),    # x ci1 p0 rest
                      (XB2, BLOBW, 128)]        # x p1 halves (both ci)
            for lo, hi, np_ in pieces:
                nc.gpsimd.dma_start(out=mt[0:np_, lo:hi], in_=blob_d[0:np_, lo:hi])

            # warm-up matmuls: bridge the gap between the weight slab landing
            # and the first x piece landing, so the PE starts the real stream
            # already un-throttled.
            wps = pw.tile([128, TC], f32, tag="warm")
            nc.tensor.matmul(wps[:], mt[:, 0:128], mt[:, 0:TC],
                             start=True, stop=True, skip_group_check=True)

            # encoder matmuls + LIF scan + running max.
            # The two channel-tile chunk loops are interleaved so the first
            # four matmul groups land on fresh PSUM banks: a matmul then
            # never needs both a DMA wait and a PSUM-recycle (DVE) wait —
            # the instruction encodings here have a single sync-wait slot.
            for l in range(NT):
                t0 = l * TC
                for ci, vb in enumerate((vb1, vb2)):
                    if l <= 2:
                        xoff = (XB0, XB1)[ci] + t0
                    else:
                        xoff = (XB2, XB3)[ci] + t0 - 1528
                    aoff = ci * 9 * 128
                    ps = pp.tile([128, TC], f32, tag="ps")
                    for k in range(9):
                        nc.tensor.matmul(
                            ps[:],
                            mt[:, aoff + k * 128:aoff + (k + 1) * 128],
                            mt[:, xoff + k:xoff + k + TC],
                            start=(k == 0),
                            stop=False,
                        )
                    nc.tensor.matmul(
                        ps[:], mt[0:36, XWB + PADT + 128 * ci:XWB + PADT + 128 * (ci + 1)],
                        mt[0:36, XWB + PAD + t0:XWB + PAD + t0 + TC],
                        start=False, stop=True,
                    )
                    # ScalarE (otherwise idle) evacuates PSUM immediately:
                    # the PE never waits on a PSUM bank, and the scan reads
                    # SBUF (faster DVE path than PSUM).
                    dsb = dp.tile([128, TC], f32, tag="dsb")
                    nc.scalar.copy(out=dsb[:], in_=ps[:])
                    init = 0.0 if l == 0 else vb[:, t0 - 1:t0]
                    nc.vector.tensor_tensor_scan(
                        vb[:, t0:t0 + TC], alpha_t[:], dsb[:], init,
                        mybir.AluOpType.mult, mybir.AluOpType.add,
                    )
                    # per-chunk running max, overlapped with the pipeline
                    # (a single end-of-kernel reduce would serialize ~10us)
                    nc.vector.tensor_reduce(
                        vmax_cols[:, 8 * ci + l:8 * ci + l + 1],
                        vb[:, t0:t0 + TC],
                        axis=mybir.AxisListType.X, op=mybir.AluOpType.max,
                    )
            for ci in range(2):
                nc.vector.tensor_reduce(
                    vmax_t[:, ci:ci + 1], vmax_cols[:, 8 * ci:8 * ci + 8],
                    axis=mybir.AxisListType.X, op=mybir.AluOpType.max,
                )
            nc.sync.dma_start(out=vmax_d[:], in_=vmax_t[:])
    # bacc legalization: split multi-sync-waits into event-semaphore chains
    # (TRN2 allows one wait per instruction), move matmul waits to ldweights.
    nc.compile()
    return nc


def _get_program():
    global _PROG
    if _PROG is None:
        _PROG = _build_program()
    return _PROG


# ------------------------------------------------------- host-side fallback
def _exact_row(x_row3, g_row, beta_c):
    """Exact fp32 drive + sequential LIF scan + first crossing for one (b,c).
    x_row3: (3, T) the three source rows, g_row: (3, 9)."""
    xp = np.pad(x_row3.astype(np.float32), ((0, 0), (PAD, PAD)))
    d = np.full((T,), np.float32(beta_c), np.float32)
    for j in range(3):
        for k in range(9):
            d += np.float32(g_row[j, k]) * xp[j, k:k + T]
    a = np.float32(ALPHA)
    oma = np.float32(OMA)
    V = np.float32(0.0)
    first = -1
    for t in range(T):
        V = a * V + oma * d[t]
        if first < 0 and V >= np.float32(THRESHOLD):
            first = t
    return first


# ------------------------------------------------------------------- kernel
def kernel(x, w3, b3, w5, b5, w9, b9, w_red, b_red,
           latency_scale, output_gates, bias, W1, b1, W2, b2):
    x = np.asarray(x, np.float32)
    g64, beta64 = _compose_g(w3, b3, w5, b5, w9, b9, w_red, b_red)
    assert np.abs(beta64).max() < 1e-30, "nonzero conv biases not supported"
    A1, A2, B1m, B2m = _build_weights(g64)
    # device layout: A[s, 9k+..] = stack of 9 (128,128) lhsT slabs
    A1f = np.ascontiguousarray(np.transpose(A1, (1, 0, 2)).reshape(128, 9 * 128))
    A2f = np.ascontiguousarray(np.transpose(A2, (1, 0, 2)).reshape(128, 9 * 128))

    x_bf = x.astype(BF16)
    xpad = np.zeros((B, C, PADT), BF16)
    xpad[:, :, PAD:PAD + T] = x_bf
    blob = np.zeros((B, 128, BLOBW), BF16)
    blob[:, :, 0:AAW] = np.concatenate([A1f, A2f], axis=1).astype(BF16)[None]
    # xw3 region (36 rows): xw3[9r+k, PAD+t] = x[s_r, t+k-4]
    for r, s in enumerate(W3ROWS):
        for k in range(9):
            lo = XWB + 8 - k
            blob[:, 9 * r + k, lo:lo + T] = x_bf[:, s, :]
    blob[:, 0:36, XWB + PADT:XWB + PADT + 128] = B1m.astype(BF16)[None]
    blob[:, 0:36, XWB + PADT + 128:XWB + PADT + 256] = B2m.astype(BF16)[None]
    blob[:, :, XB0:XB0 + 1546] = xpad[:, 0:128, 0:1546]
    blob[:, :, XB1:XB1 + 1546] = xpad[:, 128:256, 0:1546]
    blob[:, :, XB2:XB2 + 2576] = xpad[:, 0:128, 1528:4104]
    blob[:, :, XB3:XB3 + 2576] = xpad[:, 128:256, 1528:4104]

    in_maps = [dict(blob=np.ascontiguousarray(blob[i])) for i in range(NCORES)]

    nc = _get_program()
    res = run_bass_kernel_spmd(nc, in_maps, core_ids=list(range(NCORES)))
    global LAST_RESULTS
    LAST_RESULTS = res

    vmax = np.empty((B, C), np.float32)
    for i in range(NCORES):
        vm = np.asarray(res.results[i]["vmax"], np.float32)
        vmax[i, PERM1] = vm[:, 0]
        vmax[i, PERM2] = vm[:, 1]

    global LAST_VMAX
    LAST_VMAX = vmax

    # latency from Vmax; exact host recompute for near-threshold rows
    lat = np.full((B, C), np.float32(T), np.float32)
    risky = np.argwhere(vmax >= np.float32(FALLBACK_THR))
    g32 = g64.astype(np.float32)
    for b_, c_ in risky:
        srcs = [(3 * c_ + j) % 256 for j in range(3)]
        first = _exact_row(x[b_, srcs, :], g32[c_], float(beta64[c_]))
        lat[b_, c_] = np.float32(first if first >= 0 else T)

    # tiny MLP head (fp32, mirrors reference ops)
    scale = np.maximum(np.asarray(latency_scale, np.float32), np.float32(0.001))
    act = np.exp(-lat / scale).astype(np.float32)
    mixed = (act @ np.asarray(output_gates, np.float32).T
             + np.asarray(bias, np.float32)[None, :]).astype(np.float32)
    h = np.maximum(mixed @ np.asarray(W1, np.float32)
                   + np.asarray(b1, np.float32), np.float32(0)).astype(np.float32)
    raw = (h @ np.asarray(W2, np.float32)
           + np.asarray(b2, np.float32)).astype(np.float32)
    pred = np.clip(np.logaddexp(raw, np.float32(0)), np.float32(0),
                   np.float32(T)).astype(np.float32)
    return pred, lat, act


# revision 21
# speedup vs baseline: 1.1682x; 1.0029x over previous
"""Trainium2 Bass kernel for nn_ChannelLatencySeq2Value (B=8, C=256, T=4096).

Structure of the computation (derived analytically from the reference):
  * The 3 depthwise conv paths (k=3/5/9, out_per_kernel=6) followed by the
    grouped 1x1 reduce collapse into a single sparse conv:
        drive[b,c,t] = beta[c] + sum_{j<3} sum_{k<9} g[c,j,k] * x[b,(3c+j)%256, t+k-4]
    i.e. output channel c reads 3 cyclically-consecutive input channels with
    composed 9-tap kernels. g/beta are composed on the host (tiny).
  * The LIF scan V = a*V + (1-a)*drive is a first-order linear recurrence ->
    tensor_tensor_scan on VectorE (fp32 state).
  * first-spike latency only needs per-row max(V) when no neuron is near
    threshold; rows with max(V) >= 0.95 are recomputed exactly on the host
    (for the fixed reference input distribution, max V ~ 0.76, so this
    fallback never triggers; it guarantees exactness if it ever does).
  * the tiny (B,C) MLP head runs on the host in fp32 (<< 0.01% of FLOPs).

Device work per core (data-parallel over batch, 1 batch element per core):
  * encoder as bf16 matmuls on TensorE: output channels are split in two
    PSUM tiles whose input needs are covered by the two natural 128-channel
    windows of x ([0,128) and [128,256)); because gcd(3,256)=1, each window
    mod-covers 126 full output channels. 9 shift-matmuls per window
    accumulate in PSUM. The 4 boundary channels {42,85,170,213} get their
    single out-of-window tap row via a small 36-partition shift-replicated
    tile (one extra matmul per PSUM tile).
  * tensor_tensor_scan (chunked over PSUM tiles, chained via the last
    column) produces V; tensor_reduce(max) gives per-row Vmax.
"""

import numpy as np
import ml_dtypes

import concourse.bass as bass
import concourse.bacc as bacc
import concourse.mybir as mybir
from concourse.tile import TileContext
from concourse.bass_utils import run_bass_kernel_spmd


def _ensure_axon_hooks():
    # bass_utils' BASS_TRACE path imports antenv.axon_hooks, which does not
    # exist in this image; provide a no-op stub so a stray BASS_TRACE env
    # var cannot crash the kernel (tracing is then skipped gracefully).
    try:
        import antenv.axon_hooks  # noqa: F401
    except ImportError:
        import sys
        import types
        m = types.ModuleType("antenv.axon_hooks")
        m.get_axon_ntff_profile_hook = lambda: None
        m.set_axon_ntff_profile_hook = lambda h: None
        sys.modules["antenv.axon_hooks"] = m


_ensure_axon_hooks()

# ---------------------------------------------------------------- constants
B, C, T = 8, 256, 4096
OP = 6
ALPHA = float(np.exp(-1.0 / 5.0))
OMA = 1.0 - ALPHA
THRESHOLD = 1.0
TC = 512                      # time chunk (= one PSUM bank of fp32)
NT = T // TC
PAD = 4                       # conv halo (kernel width 9)
PADT = T + 2 * PAD
NCORES = 8
FALLBACK_THR = 0.95           # host exact-recompute margin for Vmax

BF16 = ml_dtypes.bfloat16

# mega-blob layout (bf16 columns, 128 partitions)
AAW = 2 * 9 * 128              # A1|A2 lhsT stacks        [0, 2304)
XWB = AAW                      # xw3 | B1 | B2 (36 rows)  [2304, 6664)
XB0 = XWB + PADT + 256         # x ci0 piece0 (xpad cols [0,1546))
XB1 = XB0 + 1546               # x ci1 piece0
XB2 = XB1 + 1546               # x ci0 piece1 (xpad cols [1528,4104))
XB3 = XB2 + 2576               # x ci1 piece1
BLOBW = XB3 + 2576

# channel -> psum-tile assignment.  Window 1 = input rows [0,128),
# window 2 = rows [128,256).  A channel c (inputs {3c+j mod 256}) is "full"
# in a window if all three input rows fall inside it.
PERM1 = list(range(0, 42)) + list(range(86, 128)) + list(range(171, 213)) + [42, 85]
PERM2 = list(range(43, 85)) + list(range(128, 170)) + list(range(214, 256)) + [170, 213]
# out-of-window tap rows used by the straddler channels, one per straddler
W3ROWS = (0, 127, 128, 255)


def _compose_g(w3, b3, w5, b5, w9, b9, w_red, b_red):
    """Collapse the 4-conv encoder into g[c,3,9] (fp64 accum) + beta[c]."""
    g = np.zeros((C, 3, 9), np.float64)
    beta = np.zeros((C,), np.float64)
    paths = [(np.asarray(w3, np.float64), np.asarray(b3, np.float64), 3),
             (np.asarray(w5, np.float64), np.asarray(b5, np.float64), 5),
             (np.asarray(w9, np.float64), np.asarray(b9, np.float64), 9)]
    wr = np.asarray(w_red, np.float64)
    for c in range(C):
        beta[c] += float(b_red[c])
        for i in range(18):
            m = c * 18 + i
            wp, bp, K = paths[m // (C * OP)]
            q = m % (C * OP)
            s = q // OP
            j = (s - 3 * c) % 256
            assert j in (0, 1, 2)
            pad = (K - 1) // 2
            w = wr[c, i, 0]
            beta[c] += w * bp[q]
            g[c, j, 4 - pad:4 + pad + 1] += w * wp[q, 0, :]
    return g, beta


def _build_weights(g):
    """Split (1-a)*g into the window lhsT stacks A1/A2 (9,128,128) and the
    straddler lhsT B1/B2 (36,128).  B row layout: p = 9*r + k where r indexes
    W3ROWS and k the shift."""
    gs = g * OMA
    A = [np.zeros((9, 128, 128), np.float64) for _ in range(2)]
    Bm = [np.zeros((36, 128), np.float64) for _ in range(2)]
    for ti, perm in enumerate((PERM1, PERM2)):
        lo = 128 * ti
        for p, c in enumerate(perm):
            for j in range(3):
                s = (3 * c + j) % 256
                if lo <= s < lo + 128:
                    A[ti][:, s - lo, p] = gs[c, j, :]
                else:
                    r = W3ROWS.index(s)
                    Bm[ti][9 * r:9 * r + 9, p] = gs[c, j, :]
    return A[0], A[1], Bm[0], Bm[1]


# ------------------------------------------------------------ device program
_PROG = None
LAST_RESULTS = None
LAST_VMAX = None


def _build_program():
    f32 = mybir.dt.float32
    bf = mybir.dt.bfloat16
    nc = bacc.Bacc(None, target_bir_lowering=False)
    # All inputs are pre-padded / pre-laid-out on the host into a single
    # mega-blob; it is loaded in consumption-priority order via SWDGE
    # (see the `pieces` list below).
    # layout: [ AA(2304) | xw3+B1+B2(4360, 36 rows) | x-ci0-p0(1546)
    #           | x-ci1-p0(1546) | x-ci0-p1(2576) | x-ci1-p1(2576) ]
    # x pieces p0/p1 overlap by 18 xpad columns so every chunk's 520-column
    # read window lies entirely inside one piece.
    blob_d = nc.declare_dram_parameter("blob", [128, BLOBW], bf, isOutput=False)
    vmax_d = nc.declare_dram_parameter("vmax", [128, 2], f32, isOutput=True)

    with TileContext(nc) as tc:
        with (
            tc.tile_pool(name="cst", bufs=1) as cst,
            tc.tile_pool(name="ps", bufs=6, space="PSUM") as pp,
            tc.tile_pool(name="pw", bufs=1, space="PSUM") as pw,
            tc.tile_pool(name="dp", bufs=3) as dp,
        ):
            mt = cst.tile([128, BLOBW], bf, tag="mt")
            alpha_t = cst.tile([128, TC], f32, tag="alpha")
            vb1 = cst.tile([128, T], f32, tag="vb1")
            vb2 = cst.tile([128, T], f32, tag="vb2")
            vmax_t = cst.tile([128, 2], f32, tag="vmax")
            vmax_cols = cst.tile([128, 16], f32, tag="vmax_cols")

            # memset on the vector engine: the scan (a tight-encoding STT
            # instruction with few sync-wait slots) then depends on alpha_t
            # via same-engine program order instead of a semaphore.
            nc.vector.memset(alpha_t[:], ALPHA)

            # loads via SWDGE (gpsimd): each call saturates the SDMA fan-out
            # (~340 GB/s) and calls issue in order on the gpsimd queue, so
            # the pieces arrive in consumption-priority order at full
            # bandwidth (HWDGE rings would either fair-share or serialize at
            # single-ring bandwidth).
            # piece order = consumption priority (the SWDGE trigger stream
            # is the bottleneck at ~0.7us/call, so small critical pieces go
            # first: warmup slab, first chunk of each x half, weights, ...)
            pieces = [(0, 514, 128),            # aa k0-3 (warmup + first MMs)
                      (XB0, XB0 + 520, 128),    # x ci0 chunk0
                      (514, AAW, 128),          # aa rest
                      (XB1, XB1 + 520, 128),    # x ci1 chunk0
                      (XWB, XB0, 36),           # xw3 + B weights
                      (XB0 + 520, XB1, 128),    # x ci0 p0 rest
                      (XB1 + 520, XB2, 128),    # x ci1 p0 rest
                      (XB2, BLOBW, 128)]        # x p1 halves (both ci)
            for lo, hi, np_ in pieces:
                nc.gpsimd.dma_start(out=mt[0:np_, lo:hi], in_=blob_d[0:np_, lo:hi])

            # warm-up matmuls: bridge the gap between the weight slab landing
            # and the first x piece landing, so the PE starts the real stream
            # already un-throttled.
            wps = pw.tile([128, TC], f32, tag="warm")
            nc.tensor.matmul(wps[:], mt[:, 0:128], mt[:, 0:TC],
                             start=True, stop=True, skip_group_check=True)

            # encoder matmuls + LIF scan + running max.
            # The two channel-tile chunk loops are interleaved so the first
            # four matmul groups land on fresh PSUM banks: a matmul then
            # never needs both a DMA wait and a PSUM-recycle (DVE) wait —
            # the instruction encodings here have a single sync-wait slot.
            for l in range(NT):
                t0 = l * TC
                for ci, vb in enumerate((vb1, vb2)):
                    if l <= 2:
                        xoff = (XB0, XB1)[ci] + t0
                    else:
                        xoff = (XB2, XB3)[ci] + t0 - 1528
                    aoff = ci * 9 * 128
                    ps = pp.tile([128, TC], f32, tag="ps")
                    for k in range(9):
                        nc.tensor.matmul(
                            ps[:],
                            mt[:, aoff + k * 128:aoff + (k + 1) * 128],
                            mt[:, xoff + k:xoff + k + TC],
                            start=(k == 0),
                            stop=False,
                        )
                    nc.tensor.matmul(
                        ps[:], mt[0:36, XWB + PADT + 128 * ci:XWB + PADT + 128 * (ci + 1)],
                        mt[0:36, XWB + PAD + t0:XWB + PAD + t0 + TC],
                        start=False, stop=True,
                    )
                    # ScalarE (otherwise idle) evacuates PSUM immediately:
                    # the PE never waits on a PSUM bank, and the scan reads
                    # SBUF (faster DVE path than PSUM).
                    dsb = dp.tile([128, TC], f32, tag="dsb")
                    nc.scalar.copy(out=dsb[:], in_=ps[:])
                    init = 0.0 if l == 0 else vb[:, t0 - 1:t0]
                    nc.vector.tensor_tensor_scan(
                        vb[:, t0:t0 + TC], alpha_t[:], dsb[:], init,
                        mybir.AluOpType.mult, mybir.AluOpType.add,
                    )
                    # per-chunk running max, overlapped with the pipeline
                    # (a single end-of-kernel reduce would serialize ~10us)
                    nc.vector.tensor_reduce(
                        vmax_cols[:, 8 * ci + l:8 * ci + l + 1],
                        vb[:, t0:t0 + TC],
                        axis=mybir.AxisListType.X, op=mybir.AluOpType.max,
                    )
            for ci in range(2):
                nc.vector.tensor_reduce(
                    vmax_t[:, ci:ci + 1], vmax_cols[:, 8 * ci:8 * ci + 8],
                    axis=mybir.AxisListType.X, op=mybir.AluOpType.max,
                )
            nc.sync.dma_start(out=vmax_d[:], in_=vmax_t[:])
    # bacc legalization: split multi-sync-waits into event-semaphore chains
    # (TRN2 allows one wait per instruction), move matmul waits to ldweights.
    nc.compile()
    return nc


def _get_program():
    global _PROG
    if _PROG is None:
        _PROG = _build_program()
    return _PROG


# ------------------------------------------------------- host-side fallback
def _exact_row(x_row3, g_row, beta_c):
    """Exact fp32 drive + sequential LIF scan + first crossing for one (b,c).
    x_row3: (3, T) the three source rows, g_row: (3, 9)."""
    xp = np.pad(x_row3.astype(np.float32), ((0, 0), (PAD, PAD)))
    d = np.full((T,), np.float32(beta_c), np.float32)
    for j in range(3):
        for k in range(9):
            d += np.float32(g_row[j, k]) * xp[j, k:k + T]
    a = np.float32(ALPHA)
    oma = np.float32(OMA)
    V = np.float32(0.0)
    first = -1
    for t in range(T):
        V = a * V + oma * d[t]
        if first < 0 and V >= np.float32(THRESHOLD):
            first = t
    return first


# ------------------------------------------------------------------- kernel
def kernel(x, w3, b3, w5, b5, w9, b9, w_red, b_red,
           latency_scale, output_gates, bias, W1, b1, W2, b2):
    x = np.asarray(x, np.float32)
    g64, beta64 = _compose_g(w3, b3, w5, b5, w9, b9, w_red, b_red)
    assert np.abs(beta64).max() < 1e-30, "nonzero conv biases not supported"
    A1, A2, B1m, B2m = _build_weights(g64)
    # device layout: A[s, 9k+..] = stack of 9 (128,128) lhsT slabs
    A1f = np.ascontiguousarray(np.transpose(A1, (1, 0, 2)).reshape(128, 9 * 128))
    A2f = np.ascontiguousarray(np.transpose(A2, (1, 0, 2)).reshape(128, 9 * 128))

    x_bf = x.astype(BF16)
    xpad = np.zeros((B, C, PADT), BF16)
    xpad[:, :, PAD:PAD + T] = x_bf
    blob = np.zeros((B, 128, BLOBW), BF16)
    blob[:, :, 0:AAW] = np.concatenate([A1f, A2f], axis=1).astype(BF16)[None]
    # xw3 region (36 rows): xw3[9r+k, PAD+t] = x[s_r, t+k-4]
    for r, s in enumerate(W3ROWS):
        for k in range(9):
            lo = XWB + 8 - k
            blob[:, 9 * r + k, lo:lo + T] = x_bf[:, s, :]
    blob[:, 0:36, XWB + PADT:XWB + PADT + 128] = B1m.astype(BF16)[None]
    blob[:, 0:36, XWB + PADT + 128:XWB + PADT + 256] = B2m.astype(BF16)[None]
    blob[:, :, XB0:XB0 + 1546] = xpad[:, 0:128, 0:1546]
    blob[:, :, XB1:XB1 + 1546] = xpad[:, 128:256, 0:1546]
    blob[:, :, XB2:XB2 + 2576] = xpad[:, 0:128, 1528:4104]
    blob[:, :, XB3:XB3 + 2576] = xpad[:, 128:256, 1528:4104]

    in_maps = [dict(blob=np.ascontiguousarray(blob[i])) for i in range(NCORES)]

    nc = _get_program()
    res = run_bass_kernel_spmd(nc, in_maps, core_ids=list(range(NCORES)))
    global LAST_RESULTS
    LAST_RESULTS = res

    vmax = np.empty((B, C), np.float32)
    for i in range(NCORES):
        vm = np.asarray(res.results[i]["vmax"], np.float32)
        vmax[i, PERM1] = vm[:, 0]
        vmax[i, PERM2] = vm[:, 1]

    global LAST_VMAX
    LAST_VMAX = vmax

    # latency from Vmax; exact host recompute for near-threshold rows
    lat = np.full((B, C), np.float32(T), np.float32)
    risky = np.argwhere(vmax >= np.float32(FALLBACK_THR))
    g32 = g64.astype(np.float32)
    for b_, c_ in risky:
        srcs = [(3 * c_ + j) % 256 for j in range(3)]
        first = _exact_row(x[b_, srcs, :], g32[c_], float(beta64[c_]))
        lat[b_, c_] = np.float32(first if first >= 0 else T)

    # tiny MLP head (fp32, mirrors reference ops)
    scale = np.maximum(np.asarray(latency_scale, np.float32), np.float32(0.001))
    act = np.exp(-lat / scale).astype(np.float32)
    mixed = (act @ np.asarray(output_gates, np.float32).T
             + np.asarray(bias, np.float32)[None, :]).astype(np.float32)
    h = np.maximum(mixed @ np.asarray(W1, np.float32)
                   + np.asarray(b1, np.float32), np.float32(0)).astype(np.float32)
    raw = (h @ np.asarray(W2, np.float32)
           + np.asarray(b2, np.float32)).astype(np.float32)
    pred = np.clip(np.logaddexp(raw, np.float32(0)), np.float32(0),
                   np.float32(T)).astype(np.float32)
    return pred, lat, act
